# revision 35
# baseline (speedup 1.0000x reference)
"""Trainium2 Bass kernel for nn_Encoder (GNN message passing, PDP-VRP encoder).

Sharding: 2 graphs per core x 8 cores. Cross-graph row scramble handled with a
ReduceScatter in global-flat row order; BatchNorm stats via moment AllReduce.
Conv in feature-major layout:
  psum[h,(j,i)] = laT.T @ E_sb (+mask fold) + wi.T@x bcast + wj.T@x bcast
  t = prelu(psum + bias) on Act (bias = BN-shift - 200, mask row adds +200)
  P = exp(t) f16; D = sum_i P (DVE fold1+fold2+red); N = sum_i P*x
  (DVE mult, Pool folds, DVE red; N-reductions deferred one group to hide
  the Pool round-trip). d/r chains staggered by half a layer; FF head part-1
  runs inside each chain's stream. E embeddings stay in SBUF (no DRAM
  round-trip); weight loads batched into a few packed DMAs.
"""
import numpy as np

B, D, NN = 16, 2, 100
N2, NA = 50, 102
H, HE, L = 128, 64, 3
SLOPE, EPS = 0.2, 1e-5
NCORE = 8
BL = 2                     # graphs per core
COLS = BL * NA * NA        # 20808 edge cols per chain per core
FLAT = B * (NA + 2 * N2)   # 3232 global flat rows
WIN = FLAT // NCORE        # 404 rows per core window
ECH = 1536                 # embed streaming chunk

_CACHE = {}


def _chunks_full():
    return [(j, 5) for j in range(0, 100, 5)] + [(100, 2)]


def _chunks_sub():
    return [(j, 10) for j in range(0, 50, 10)]


def _groups(chunks, n=3):
    return [chunks[i:i + n] for i in range(0, len(chunks), n)]


def build(emulate_collectives=False):
    import concourse.bass as bass
    import concourse.bacc as bacc
    import concourse.tile as tile
    import concourse.mybir as mybir
    from concourse import masks

    dt = mybir.dt
    F32, F16 = dt.float32, dt.float16
    AF = mybir.ActivationFunctionType
    OP = mybir.AluOpType
    AX = mybir.AxisListType

    nc = bacc.Bacc("TRN2", target_bir_lowering=False, debug=False,
                   num_devices=NCORE)

    def din(name, shape, d=F32):
        return nc.dram_tensor(name, shape, d, kind="ExternalInput").ap()

    dsT = din("dsT", [5, BL * NA])
    pkinT = din("pkinT", [10, BL * N2])
    nat_pack = din("nat_pack", [BL * N2, 23])      # dep|pk|dl natural-layout
    eT = {c: din(f"eT_{c}", [2, COLS], F16) for c in "dr"}
    e_nat = {c: din(f"e_nat_{c}", [128, 163 * 3]) for c in "dr"}
    m_in = {c: din(f"m_{c}", [1, COLS], F16) for c in "dr"}
    W012 = din("W012", [10, 384])                  # W1 | W0 | W2 columns
    W34 = din("W34", [2, 128])                     # W3 | W4
    bcol = din("bcol", [128, 14])                  # packed bias columns
    wv_all = din("wv_all", [128, 9 * H])           # (k,l) major
    wi_all = din("wi_all", [128, 9 * H])
    wj_all = din("wj_all", [128, 9 * H])
    we_all = din("we_all", [64, 9 * H])
    ff_w1 = din("ff_w1", [H, H])
    ff_w2 = din("ff_w2", [H, H])

    o_out = {c: nc.dram_tensor(f"o_{c}", [BL, NA, H], F32,
                               kind="ExternalOutput").ap() for c in "dr"}

    rs_in = {(c, l): nc.dram_tensor(f"rsi_{c}{l}", [FLAT, H], F32).ap()
             for c in "dr" for l in range(L)}
    rs_out = {(c, l): nc.dram_tensor(f"rso_{c}{l}", [WIN, H], F32).ap()
              for c in "dr" for l in range(L)}
    ar1_i = nc.dram_tensor("ar1_i", [128, 16], F32).ap()
    ar1_o = nc.dram_tensor("ar1_o", [128, 16], F32).ap()
    ar2_i = nc.dram_tensor("ar2_i", [128, 8], F32).ap()
    ar2_o = nc.dram_tensor("ar2_o", [128, 8], F32).ap()
    GRP = [list(range(NCORE))]

    import contextlib
    with tile.TileContext(nc) as tc, contextlib.ExitStack() as ctx:
        cpool = ctx.enter_context(tc.tile_pool(name="const", bufs=1))
        wk = ctx.enter_context(tc.tile_pool(name="work", bufs=3))
        xpool = ctx.enter_context(tc.tile_pool(name="xt", bufs=6))
        fpool = ctx.enter_context(tc.tile_pool(name="f16", bufs=3))
        ps_b = ctx.enter_context(tc.tile_pool(name="psb", bufs=2, space="PSUM"))
        ps_s = ctx.enter_context(tc.tile_pool(name="pss", bufs=2, space="PSUM"))

        def ctile(shape, d, tag):
            return cpool.tile(shape, d, tag=tag, name=tag)

        ident = ctile([128, 128], F32, "ident")
        masks.make_identity(nc, ident[:])

        # ---- batched constant loads ----
        W012s = ctile([10, 384], F32, "W012s")
        nc.sync.dma_start(W012s[:], W012[:])
        W1s = W012s[:, 0:128]
        W0s = W012s[0:5, 128:256]
        W2s = W012s[0:5, 256:384]
        W34s = ctile([2, 128], F32, "W34s")
        nc.sync.dma_start(W34s[:], W34[:])
        Wes = {"d": W34s[:, 0:64], "r": W34s[:, 64:128]}
        W34h = ctile([2, 128], F16, "W34h")
        nc.vector.tensor_copy(W34h[:], W34s[:])
        Wesh = {"d": W34h[:, 0:64], "r": W34h[:, 64:128]}
        bcs = ctile([128, 14], F32, "bcs")
        nc.sync.dma_start(bcs[:], bcol[:])
        # col order: b0_g,b0_b,b1_g,b1_b,b2_g,b2_b,ff_b1,ff_b2,bn_g,bn_b,
        #            b3_g,b3_b,b4_g,b4_b (last 4 rows 0:64)
        gcol = {f"b{i}_g": bcs[0:128, 2 * i:2 * i + 1] for i in range(3)}
        gcol.update({f"b{i}_b": bcs[0:128, 2 * i + 1:2 * i + 2]
                     for i in range(3)})
        ffb1c, ffb2c = bcs[:, 6:7], bcs[:, 7:8]
        bngc, bnbc = bcs[:, 8:9], bcs[:, 9:10]
        gcol["b3_g"] = bcs[0:64, 10:11]
        gcol["b3_b"] = bcs[0:64, 11:12]
        gcol["b4_g"] = bcs[0:64, 12:13]
        gcol["b4_b"] = bcs[0:64, 13:14]

        wvs = ctile([128, 9 * H], F32, "wvs")
        nc.sync.dma_start(wvs[:], wv_all[:])
        KIDX = {("a", 0): 0, ("a", 1): 1, ("a", 2): 2,
                ("p", 0): 3, ("p", 1): 4, ("p", 2): 5,
                ("d", 0): 6, ("d", 1): 7, ("d", 2): 8}

        def wsl(t, k, l):
            i = KIDX[(k, l)]
            return t[:, i * H:(i + 1) * H]

        wi_s, wj_s = {}, {}
        for src_dram, dst in ((wi_all, wi_s), (wj_all, wj_s)):
            wtmp = wk.tile([128, 9 * H], F32, tag="wtmp", bufs=1, name="wtmp")
            nc.sync.dma_start(wtmp[:], src_dram[:])
            for k in "apd":
                for l in range(L):
                    nm = "wir" if dst is wi_s else "wjr"
                    tir = ctile([H, H], dt.float32r, f"{nm}{k}{l}")
                    nc.vector.tensor_copy(tir[:], wsl(wtmp, k, l))
                    dst[(k, l)] = tir

        with tc.tile_critical():
            pid = nc.gpsimd.partition_id()

        zero128 = ctile([128, 128], F32, "zero")
        nc.vector.memset(zero128[:], 0.0)

        # ---------------- embeddings & stats ----------------
        stats = ctile([128, 16], F32, "stats")
        nc.vector.memset(stats[:], 0.0)

        def evac(ps_ap, hh, wid, tag, d=F32):
            t = wk.tile([hh, wid], d, tag=tag, name=tag)
            nc.scalar.copy(t[:], ps_ap)
            return t

        natp = wk.tile([BL * N2, 23], F32, tag="natp", name="natp")
        nc.sync.dma_start(natp[:], nat_pack[:])

        def moments(nat_ap, rows, fdim):
            ps = ps_s.tile([fdim, fdim + 1], F32, tag="pss", name="psm")
            nc.tensor.matmul(ps[:], nat_ap[0:rows, 0:fdim], nat_ap[0:rows, :],
                             start=True, stop=True)
            return evac(ps[:], fdim, fdim + 1, f"mom{fdim}")

        def w_transpose(w, kdim, hh):
            ps = ps_s.tile([hh, kdim], F32, tag="pss", name="pswt")
            nc.tensor.transpose(ps[:], w, ident[:kdim, :kdim])
            return evac(ps[:], hh, kdim, "wT")

        def embed_stats(mom, Ws, kdim, hh, scol):
            ps = ps_s.tile([hh, 1], F32, tag="pss", name="pse1")
            nc.tensor.matmul(ps[:], Ws, mom[:, kdim:kdim + 1],
                             start=True, stop=True)
            nc.vector.tensor_copy(stats[0:hh, scol:scol + 1], ps[:])
            ps2 = ps_s.tile([hh, kdim], F32, tag="pss", name="pse2")
            nc.tensor.matmul(ps2[:], Ws, mom[:, 0:kdim],
                             start=True, stop=True)
            G2 = evac(ps2[:], hh, kdim, "G2")
            WT = w_transpose(Ws, kdim, hh)
            prod = wk.tile([hh, kdim], F32, tag="prod", name="prod")
            nc.vector.tensor_tensor(prod[:], G2[:], WT[:], op=OP.mult)
            nc.vector.tensor_reduce(stats[0:hh, scol + 1:scol + 2], prod[:],
                                    axis=AX.X, op=OP.add)

        # nat_pack: cols 0:6 dep (rows 0:BL*D), 6:17 pk, 17:23 dl
        embed_stats(moments(natp[0:BL * D, 0:6], BL * D, 5), W0s, 5, H, 0)
        embed_stats(moments(natp[:, 6:17], BL * N2, 10), W1s, 10, H, 2)
        embed_stats(moments(natp[:, 17:23], BL * N2, 5), W2s, 5, H, 4)

        for ci, c in enumerate("dr"):
            na = wk.tile([128, 163 * 3], F32, tag="enat", bufs=1, name="enat")
            nc.sync.dma_start(na[:], e_nat[c][:])
            nav = na[:].rearrange("p (n c) -> p n c", n=163)
            ps = ps_s.tile([2, 3], F32, tag="pss", name="psen")
            for n in range(163):
                nc.tensor.matmul(ps[:], nav[:, n, 0:2], nav[:, n, :],
                                 start=(n == 0), stop=(n == 162))
            mom = evac(ps[:], 2, 3, "mome")
            embed_stats(mom, Wes[c], 2, HE, 6 + 2 * ci)

        # ---- edge embedding into SBUF: E_sb rows 0..63 raw z; row 64 mask ----
        E_sb = {}
        for ei, c in enumerate("dr"):
            Et = cpool.tile([65, COLS], F16, tag=f"Esb{c}", name=f"Esb{c}")
            E_sb[c] = Et
            EQ = 2 * ECH  # 3072: eT staging block, multiple of ECH
            ets = None
            for ci2, c0 in enumerate(range(0, COLS, ECH)):
                CH = min(ECH, COLS - c0)
                if c0 % EQ == 0:
                    ets = wk.tile([2, EQ], F16, tag="ets", bufs=2, name="ets")
                    nc.sync.dma_start(ets[:, 0:min(EQ, COLS - c0)],
                                      eT[c][:, c0:c0 + min(EQ, COLS - c0)])
                eo = c0 % EQ
                psg = ps_b.tile([128, 1536], F32, tag="psg", name="psge")
                for k in range((CH + 511) // 512):
                    w = min(512, CH - k * 512)
                    nc.tensor.matmul(psg[0:64, k * 512:k * 512 + w],
                                     Wesh[c],
                                     ets[:, eo + k * 512:eo + k * 512 + w],
                                     start=True, stop=True)
                if ci2 % 2 == 0:
                    nc.scalar.copy(Et[0:64, c0:c0 + CH], psg[0:64, 0:CH])
                else:
                    nc.vector.tensor_copy(Et[0:64, c0:c0 + CH],
                                          psg[0:64, 0:CH])
            nc.sync.dma_start(Et[64:65, :], m_in[c][:])

        zrep = zero128[:].unsqueeze(1).broadcast_to([128, 25, 128])
        for key, t in rs_in.items():
            nc.scalar.dma_start(t[0:3200, :].rearrange("(a b) h -> b a h", b=128),
                                zrep)
            nc.scalar.dma_start(t[3200:FLAT, :], zero128[0:32, :])

        nc.sync.dma_start(ar1_i[:], stats[:])
        if emulate_collectives:
            nc.sync.dma_start(ar1_o[:], ar1_i[:])
        else:
            nc.gpsimd.collective_compute("AllReduce", OP.add, replica_groups=GRP,
                                         ins=[ar1_i], outs=[ar1_o])
        sts = ctile([128, 16], F32, "sts")
        nc.sync.dma_start(sts[:], ar1_o[:])

        def bn_vecs(src, scol, n, gc, bc, hh, tag):
            inv = 1.0 / n
            m = wk.tile([hh, 1], F32, tag=f"m{tag}", name=f"m{tag}")
            nc.vector.tensor_scalar_mul(m[:], src[0:hh, scol:scol + 1], inv)
            v = wk.tile([hh, 1], F32, tag=f"v{tag}", name=f"v{tag}")
            nc.vector.tensor_scalar_mul(v[:], src[0:hh, scol + 1:scol + 2], inv)
            msq = wk.tile([hh, 1], F32, tag=f"q{tag}", name=f"q{tag}")
            nc.vector.tensor_tensor(msq[:], m[:], m[:], op=OP.mult)
            nc.vector.tensor_tensor(v[:], v[:], msq[:], op=OP.subtract)
            nc.vector.tensor_scalar_add(v[:], v[:], EPS)
            sd = wk.tile([hh, 1], F32, tag=f"s{tag}", name=f"s{tag}")
            nc.scalar.activation(sd[:], v[:], AF.Sqrt)
            rsd = wk.tile([hh, 1], F32, tag=f"r{tag}", name=f"r{tag}")
            nc.vector.reciprocal(rsd[:], sd[:])
            sc = ctile([hh, 1], F32, f"sc{tag}")
            nc.vector.tensor_tensor(sc[:], rsd[:], gc, op=OP.mult)
            sh = ctile([hh, 1], F32, f"sh{tag}")
            nc.vector.tensor_tensor(sh[:], m[:], sc[:], op=OP.mult)
            nc.vector.tensor_tensor(sh[:], bc, sh[:], op=OP.subtract)
            return sc, sh

        sc0, sh0 = bn_vecs(sts, 0, B * D, gcol["b0_g"], gcol["b0_b"], H, "b0")
        sc1, sh1 = bn_vecs(sts, 2, B * N2, gcol["b1_g"], gcol["b1_b"], H, "b1")
        sc2, sh2 = bn_vecs(sts, 4, B * N2, gcol["b2_g"], gcol["b2_b"], H, "b2")
        sce, she = {}, {}
        sce["d"], she["d"] = bn_vecs(sts, 6, B * NA * NA, gcol["b3_g"],
                                     gcol["b3_b"], HE, "b3")
        sce["r"], she["r"] = bn_vecs(sts, 8, B * NA * NA, gcol["b4_g"],
                                     gcol["b4_b"], HE, "b4")

        # lhsT (65 rows: 64 e-rows scaled by BN, row 64 = +200 mask weight),
        # bias column = we.T @ she - 200, and its f16 row for the K=1 matmul.
        wes_f = wk.tile([64, 9 * H], F32, tag="wtmp", bufs=1, name="wes_f")
        nc.sync.dma_start(wes_f[:], we_all[:])
        lhsT_aug, bias_col = {}, {}
        for c in "dr":
            for k in "apd":
                for l in range(L):
                    t = ctile([65, H], F16, f"la{c}{k}{l}")
                    nc.vector.tensor_scalar(t[0:64, :], wsl(wes_f, k, l),
                                            sce[c][:], None, op0=OP.mult)
                    nc.vector.memset(t[64:65, :], 200.0)
                    lhsT_aug[(c, k, l)] = t
                    ps = ps_s.tile([H, 1], F32, tag="pss", name="pscc")
                    nc.tensor.matmul(ps[:], wsl(wes_f, k, l), she[c][:],
                                     start=True, stop=True)
                    bc = ctile([H, 1], F32, f"bc{c}{k}{l}")
                    nc.vector.tensor_scalar_add(bc[:], ps[:], -200.0)
                    bias_col[(c, k, l)] = bc

        # node embeddings -> xT0
        xT0 = xpool.tile([H, BL * NA], F32, tag="xT", name="xT0")
        dsTs = wk.tile([5, BL * NA], F32, tag="dsTs", name="dsTs")
        nc.sync.dma_start(dsTs[:], dsT[:])
        dsv = dsTs[:].rearrange("p (g n) -> p g n", g=BL)
        x0v = xT0[:].rearrange("p (g n) -> p g n", g=BL)
        ps = ps_s.tile([H, BL * D], F32, tag="pss", name="psx0")
        nc.tensor.matmul(ps[:], W0s, dsv[:, :, 0:D], start=True, stop=True)
        nc.vector.tensor_scalar(
            x0v[:, :, 0:D], ps[:].rearrange("p (g n) -> p g n", g=BL),
            sc0[:], sh0[:], op0=OP.mult, op1=OP.add)
        pkt = wk.tile([10, BL * N2], F32, tag="pkt", name="pkt")
        nc.sync.dma_start(pkt[:], pkinT[:])
        ps = ps_s.tile([H, BL * N2], F32, tag="pss", name="psx1")
        nc.tensor.matmul(ps[:], W1s, pkt[:], start=True, stop=True)
        nc.vector.tensor_scalar(
            x0v[:, :, D:D + N2], ps[:].rearrange("p (g n) -> p g n", g=BL),
            sc1[:], sh1[:], op0=OP.mult, op1=OP.add)
        ps = ps_s.tile([H, BL * N2], F32, tag="pss", name="psx2")
        nc.tensor.matmul(ps[:], W2s, dsv[:, :, D + N2:NA],
                         start=True, stop=True)
        nc.vector.tensor_scalar(
            x0v[:, :, D + N2:NA], ps[:].rearrange("p (g n) -> p g n", g=BL),
            sc2[:], sh2[:], op0=OP.mult, op1=OP.add)

        # ---------------- conv layers ----------------
        grp_ctr = [0]
        unit_ctr = [0]
        defq = []   # deferred-by-one-group ops (cross-engine latency hiding)

        def flush_defq():
            for fn in defq:
                fn()
            defq.clear()

        def mm_evac(w, rhs_ap, wid, tag):
            ps = ps_s.tile([H, wid], F32, tag="pss", name="psmm")
            nc.tensor.matmul(ps[:], w, rhs_ap, start=True, stop=True)
            t = xpool.tile([H, wid], dt.float32r, tag=tag, bufs=2, name=tag)
            nc.scalar.copy(t[:], ps[:])
            th = xpool.tile([H, wid], F16, tag=tag + "h", bufs=2,
                            name=tag + "h")
            nc.vector.tensor_copy(th[:], ps[:])
            return t, th

        def conv_units(c, l, xTin):
            xv = xTin[:].rearrange("p (g n) -> p g n", g=BL)
            xall, xallh = mm_evac(wsl(wvs, "a", l), xTin[:], BL * NA, "xa")
            pick, pickh = mm_evac(wsl(wvs, "p", l), xv[:, :, D:D + N2],
                                  BL * N2, "xp")
            deli, delih = mm_evac(wsl(wvs, "d", l), xv[:, :, D + N2:NA],
                                  BL * N2, "xd")
            Ev = E_sb[c][:].rearrange("p (g j i) -> p g j i", g=BL, j=NA)
            cfg = [("a", xall, xallh, NA, _chunks_full()),
                   ("p", pick, pickh, N2, _chunks_sub()),
                   ("d", deli, delih, N2, _chunks_sub())]
            units = []
            for k_, xk_, xkh_, S_, chunks_ in cfg:
                for g_ in range(BL):
                    units.append((k_, xk_, xkh_, S_, chunks_, g_))

            def emit_unit(u):
                k, xk, xkh, S, chunks, g = u
                ilen = S
                la = lhsT_aug[(c, k, l)]
                bc = bias_col[(c, k, l)]
                wi, wjt = wi_s[(k, l)], wj_s[(k, l)]
                if k == "a":
                    esrc = Ev[0:65, g]            # [65, NA, NA]
                elif k == "p":
                    esrc = Ev[0:65, g, D:D + N2, D:D + N2]
                else:
                    esrc = Ev[0:65, g, D + N2:NA, D + N2:NA]
                Dt = wk.tile([H, S], F16, tag="Dt", name="Dt")
                Nt = wk.tile([H, S], F16, tag="Nt", name="Nt")
                for grp in _groups(chunks):
                    gi = grp_ctr[0]
                    grp_ctr[0] += 1
                    flush_defq()   # prior group's deferred reds first in queue
                    psg = ps_b.tile([128, 1536], F32, tag="psg", name="psg")
                    for ki, (j0, nj) in enumerate(grp):
                        nc.tensor.matmul(
                            psg[:, ki * 512:ki * 512 + nj * ilen],
                            la[:], esrc[:, j0:j0 + nj, :],
                            start=True, stop=False)
                    for ki, (j0, nj) in enumerate(grp):
                        a_rhs = xk[:, g * S + j0:g * S + j0 + nj]\
                            .unsqueeze(2).broadcast_to([H, nj, ilen])
                        nc.tensor.matmul(
                            psg[:, ki * 512:ki * 512 + nj * ilen],
                            wi[:], a_rhs, start=False, stop=False)
                    b_base = xk[:, g * S:g * S + ilen]
                    for ki, (j0, nj) in enumerate(grp):
                        b_rhs = b_base.unsqueeze(1)\
                            .broadcast_to([H, nj, ilen])
                        nc.tensor.matmul(
                            psg[:, ki * 512:ki * 512 + nj * ilen],
                            wjt[:], b_rhs, start=False, stop=True)
                    fd = (len(grp) - 1) * 512 + grp[-1][1] * ilen
                    tg = fpool.tile([128, 1536], F16, tag="tg", bufs=3,
                                    name="tg")
                    nc.scalar.activation(tg[:, 0:fd], psg[:, 0:fd],
                                         AF.Prelu, bias=bc[:], alpha=SLOPE)
                    Pg = fpool.tile([128, 1536], F16, tag="Pg", bufs=3, name="Pg")
                    nc.scalar.activation(Pg[:, 0:fd], tg[:, 0:fd], AF.Exp)
                    wg = fpool.tile([128, 1536], F16, tag="wg", bufs=3,
                                    name="wg")
                    xkb = xkh[:, g * S:g * S + ilen]
                    runs = []
                    for ki, (j0, nj) in enumerate(grp):
                        if runs and runs[-1][2] == nj:
                            runs[-1][1] += 1
                        else:
                            runs.append([ki, 1, nj, j0])
                    hl = ilen // 2          # 51 or 25
                    ql = hl // 2            # 25 or 12
                    pf = fpool.tile([128, 768], F16, tag="pf", bufs=3,
                                    name="pf")
                    wf = fpool.tile([128, 768], F16, tag="wf", bufs=3,
                                    name="wf")
                    pq = fpool.tile([128, 384], F16, tag="pq", bufs=3,
                                    name="pq")
                    wq = fpool.tile([128, 384], F16, tag="wq", bufs=3,
                                    name="wq")
                    reds = []
                    for k0, nk, nj, j0r in runs:
                        base = Pg[:, k0 * 512:(k0 + nk) * 512]
                        wbase = wg[:, k0 * 512:(k0 + nk) * 512]
                        pv4 = base.rearrange("p (k r) -> p k r", k=nk)\
                            [:, :, 0:nj * ilen].rearrange(
                            "p k (j i) -> p k j i", i=ilen)
                        wv4 = wbase.rearrange("p (k r) -> p k r", k=nk)\
                            [:, :, 0:nj * ilen].rearrange(
                            "p k (j i) -> p k j i", i=ilen)
                        xb4 = xkb.unsqueeze(1).unsqueeze(1)\
                            .broadcast_to([H, nk, nj, ilen])
                        nc.vector.tensor_tensor(wv4, pv4, xb4, op=OP.mult)
                        pf4 = pf[:, k0 * 256:k0 * 256 + nk * nj * hl]\
                            .rearrange("p (k j i) -> p k j i", k=nk, j=nj)
                        wf4 = wf[:, k0 * 256:k0 * 256 + nk * nj * hl]\
                            .rearrange("p (k j i) -> p k j i", k=nk, j=nj)
                        # fold1: i halves
                        nc.vector.tensor_tensor(
                            pf4, pv4[:, :, :, 0:hl],
                            pv4[:, :, :, hl:2 * hl], op=OP.add)
                        nc.gpsimd.tensor_tensor(
                            wf4, wv4[:, :, :, 0:hl],
                            wv4[:, :, :, hl:2 * hl], op=OP.add)
                        # fold2: halves of hl (ql cols), leftover col if odd
                        pq4 = pq[:, k0 * 128:k0 * 128 + nk * nj * ql]\
                            .rearrange("p (k j i) -> p k j i", k=nk, j=nj)
                        wq4 = wq[:, k0 * 128:k0 * 128 + nk * nj * ql]\
                            .rearrange("p (k j i) -> p k j i", k=nk, j=nj)
                        nc.vector.tensor_tensor(
                            pq4, pf4[:, :, :, 0:ql],
                            pf4[:, :, :, ql:2 * ql], op=OP.add)
                        nc.gpsimd.tensor_tensor(
                            wq4, wf4[:, :, :, 0:ql],
                            wf4[:, :, :, ql:2 * ql], op=OP.add)
                        dv = Dt[:, j0r:j0r + nk * nj].rearrange(
                            "p (k j) -> p k j", k=nk)
                        nv = Nt[:, j0r:j0r + nk * nj].rearrange(
                            "p (k j) -> p k j", k=nk)
                        with nc.allow_low_precision("fp16 softmax sums"):
                            nc.vector.tensor_reduce(dv, pq4, axis=AX.X,
                                                    op=OP.add)
                            if 2 * ql != hl:
                                nc.vector.tensor_tensor(
                                    dv, dv, pf4[:, :, :, 2 * ql], op=OP.add)

                        def red_n(nv=nv, wq4=wq4, wf4=wf4, ql=ql, hl=hl):
                            with nc.allow_low_precision("fp16 softmax sums"):
                                nc.vector.tensor_reduce(nv, wq4, axis=AX.X,
                                                        op=OP.add)
                                if 2 * ql != hl:
                                    nc.vector.tensor_tensor(
                                        nv, nv, wf4[:, :, :, 2 * ql],
                                        op=OP.add)
                        reds.append(red_n)
                    defq.extend(reds)

                def epilogue(k=k, g=g, S=S, Dt=Dt, Nt=Nt):
                    Di = wk.tile([H, S], F32, tag="Di", name="Di")
                    nc.vector.reciprocal(Di[:], Dt[:])
                    og = wk.tile([H, S], F32, tag="og", name="og")
                    nc.vector.tensor_tensor(og[:], Nt[:], Di[:], op=OP.mult)
                    pst = ps_s.tile([S, H], F32, tag="pss", name="psout")
                    nc.tensor.transpose(pst[:], og[:], ident[:])
                    nat2 = wk.tile([S, H], F32, tag="nat2", name="nat2")
                    nc.vector.tensor_copy(nat2[:], pst[:])
                    unit_ctr[0] += 1
                    if k == "a":
                        off = pid * (BL * NA) + g * NA
                    elif k == "p":
                        off = pid * (BL * N2) + g * N2 + B * NA
                    else:
                        off = pid * (BL * N2) + g * N2 + B * (NA + N2)
                    nc.gpsimd.dma_start(rs_in[(c, l)][bass.ds(off, S), :],
                                        nat2[:])
                defq.append(epilogue)

            def emit_rs():
                flush_defq()
                if emulate_collectives:
                    nc.sync.dma_start(rs_out[(c, l)][:], rs_in[(c, l)][0:WIN, :])
                else:
                    nc.gpsimd.collective_compute(
                        "ReduceScatter", OP.add, replica_groups=GRP,
                        ins=[rs_in[(c, l)]], outs=[rs_out[(c, l)]])
            return [lambda u=u: emit_unit(u) for u in units] + [emit_rs]

        def assemble(c, l):
            xn = xpool.tile([H, BL * NA], F32, tag="xT", name="xn")
            for g in range(BL):
                n1 = wk.tile([128, H], F32, tag="asm", name="asm1")
                nc.sync.dma_start(n1[:],
                                  rs_out[(c, l)][g * 202:g * 202 + 128, :])
                n2 = wk.tile([128, H], F32, tag="asm", name="asm2")
                nc.sync.dma_start(
                    n2[0:74, :], rs_out[(c, l)][g * 202 + 128:g * 202 + 202, :])
                p1 = ps_s.tile([H, 128], F32, tag="pss", name="psa1")
                nc.tensor.transpose(p1[:], n1[:], ident[:])
                p2 = ps_s.tile([H, 128], F32, tag="pss", name="psa2")
                nc.tensor.transpose(p2[:, 0:74], n2[0:74, :],
                                    ident[0:74, 0:74])
                # cols: 0:102 xa | 102:202 xp,xd -> add onto nodes 2..102
                xb1 = wk.tile([H, 128], F32, tag="xb1", name="xb1")
                nc.vector.tensor_copy(xb1[:], p1[:])
                nc.vector.tensor_copy(xn[:, g * NA:g * NA + D], xb1[:, 0:D])
                nc.vector.tensor_tensor(xn[:, g * NA + D:g * NA + D + 26],
                                        xb1[:, D:D + 26], xb1[:, 102:128],
                                        op=OP.add)
                nc.vector.tensor_tensor(xn[:, g * NA + D + 26:g * NA + NA],
                                        xb1[:, D + 26:NA], p2[:, 0:74],
                                        op=OP.add)
            return xn

        # FF part-1 consts (loaded early so stream FF thunks can run)
        ffw1 = ctile([H, H], F32, "ffw1")
        nc.sync.dma_start(ffw1[:], ff_w1[:])
        ffw2 = ctile([H, H], F32, "ffw2")
        nc.sync.dma_start(ffw2[:], ff_w2[:])
        zt = {}
        st2 = ctile([128, 8], F32, "st2")
        nc.vector.memset(st2[:], 0.0)

        def ff_part1(c):
            ci = 0 if c == "d" else 1
            ps1 = ps_s.tile([H, BL * NA], F32, tag="pss", name="psf1")
            nc.tensor.matmul(ps1[:], ffw1[:], xT[c][:], start=True, stop=True)
            r = wk.tile([H, BL * NA], F32, tag="ffr", name="ffr")
            nc.scalar.activation(r[:], ps1[:], AF.Relu, bias=ffb1c)
            ps2 = ps_s.tile([H, BL * NA], F32, tag="pss", name="psf2")
            nc.tensor.matmul(ps2[:], ffw2[:], r[:], start=True, stop=True)
            z = xpool.tile([H, BL * NA], F32, tag="zt", bufs=2, name="zt")
            nc.vector.scalar_tensor_tensor(z[:], ps2[:], ffb2c, xT[c][:],
                                           op0=OP.add, op1=OP.add)
            zt[c] = z
            nc.vector.tensor_reduce(st2[:, 2 * ci:2 * ci + 1], z[:],
                                    axis=AX.X, op=OP.add)
            sq = fpool.tile([H, BL * NA], F16, tag="sq", name="sq")
            nc.scalar.activation(sq[:], z[:], AF.Square,
                                 accum_out=st2[:, 2 * ci + 1:2 * ci + 2])

        xT = {"d": xT0, "r": xT0}

        def stream(c):
            # per layer: prep(mm_evac) + 6 units + rs + assemble = 9 thunks
            for l in range(L):
                holder = {}

                def prep(l=l, c=c, holder=holder):
                    holder["units"] = conv_units(c, l, xT[c])
                yield prep
                for i in range(7):
                    yield (lambda i=i, holder=holder: holder["units"][i]())
                yield (lambda c=c, l=l: xT.__setitem__(c, assemble(c, l)))
            yield (lambda c=c: ff_part1(c))

        sd, sr = stream("d"), stream("r")
        for _ in range(5):          # stagger chains by ~half a layer
            next(sd)()
        nd = nr = False
        while not (nd and nr):
            if not nd:
                try:
                    next(sd)()
                except StopIteration:
                    nd = True
            if not nr:
                try:
                    next(sr)()
                except StopIteration:
                    nr = True

        # ---------------- FF head (part 2: stats exchange + BN) ---------
        nc.sync.dma_start(ar2_i[:], st2[:])
        if emulate_collectives:
            nc.sync.dma_start(ar2_o[:], ar2_i[:])
        else:
            nc.gpsimd.collective_compute("AllReduce", OP.add, replica_groups=GRP,
                                         ins=[ar2_i], outs=[ar2_o])
        st2o = ctile([128, 8], F32, "st2o")
        nc.sync.dma_start(st2o[:], ar2_o[:])
        for ci, c in enumerate("dr"):
            sc, sh = bn_vecs(st2o, 2 * ci, B * NA, bngc, bnbc, H, f"ff{ci}")
            oT = wk.tile([H, BL * NA], F32, tag="oT", name="oT")
            nc.vector.tensor_scalar(oT[:], zt[c][:], sc[:], sh[:],
                                    op0=OP.mult, op1=OP.add)
            for g in range(BL):
                pso = ps_s.tile([NA, H], F32, tag="pss", name="pso")
                nc.tensor.transpose(pso[:], oT[:, g * NA:(g + 1) * NA],
                                    ident[:])
                on = wk.tile([NA, H], F32, tag="on", name="on")
                nc.scalar.copy(on[:], pso[:])
                nc.sync.dma_start(o_out[c][g], on[:])

    nc.compile()
    return nc


def _prep_core(inputs, c):
    sl = slice(2 * c, 2 * c + 2)
    x = np.asarray(inputs["x"])[sl]
    dem = np.asarray(inputs["demand"])[sl]
    tw = np.asarray(inputs["time_window"])[sl]
    ds = np.concatenate([x, dem, tw], -1).astype(np.float32)
    dsT = np.ascontiguousarray(ds.transpose(2, 0, 1).reshape(5, BL * NA))
    pkin = np.concatenate([ds[:, D:D + N2], ds[:, D + N2:NA]], -1)
    pkinT = np.ascontiguousarray(pkin.transpose(2, 0, 1).reshape(10, BL * N2))
    nat_pack = np.ones((BL * N2, 23), np.float32)
    nat_pack[:, 0:6] = 1.0
    nat_pack[:BL * D, 0:5] = ds[:, :D].reshape(BL * D, 5)
    nat_pack[BL * D:, 0:6] = 0.0
    nat_pack[:, 6:16] = pkin.reshape(BL * N2, 10)
    nat_pack[:, 16] = 1.0
    nat_pack[:, 17:22] = ds[:, D + N2:NA].reshape(BL * N2, 5)
    nat_pack[:, 22] = 1.0
    im = dict(dsT=dsT, pkinT=pkinT, nat_pack=nat_pack)
    for c2, key_e, key_m in (("d", "edge_attr_d", "mask_adjacency_d"),
                             ("r", "edge_attr_r", "mask_adjacency_r")):
        ea = np.asarray(inputs[key_e])[sl].reshape(BL, NA, NA, 2)
        im[f"eT_{c2}"] = np.ascontiguousarray(
            ea.transpose(3, 0, 2, 1).reshape(2, COLS)).astype(np.float16)
        tmp = np.zeros((163 * 128, 3), np.float32)
        tmp[:BL * NA * NA, :2] = ea.reshape(BL * NA * NA, 2)
        tmp[:BL * NA * NA, 2] = 1.0
        im[f"e_nat_{c2}"] = np.ascontiguousarray(
            tmp.reshape(163, 128, 3).transpose(1, 0, 2).reshape(128, 489))
        mm = np.asarray(inputs[key_m])[sl].reshape(BL, NA, NA)
        im[f"m_{c2}"] = np.ascontiguousarray(
            mm.transpose(0, 2, 1).reshape(1, COLS)).astype(np.float16)
    W0 = np.asarray(inputs["W0"], np.float32)
    W1 = np.asarray(inputs["W1"], np.float32)
    W2 = np.asarray(inputs["W2"], np.float32)
    W012 = np.zeros((10, 384), np.float32)
    W012[0:10, 0:128] = W1
    W012[0:5, 128:256] = W0
    W012[0:5, 256:384] = W2
    im["W012"] = W012
    W34 = np.zeros((2, 128), np.float32)
    W34[:, 0:64] = np.asarray(inputs["W3"], np.float32)
    W34[:, 64:128] = np.asarray(inputs["W4"], np.float32)
    im["W34"] = W34
    bcol = np.zeros((128, 14), np.float32)
    for i in range(3):
        bcol[:, 2 * i] = np.asarray(inputs[f"b{i}_g"], np.float32)
        bcol[:, 2 * i + 1] = np.asarray(inputs[f"b{i}_b"], np.float32)
    bcol[:, 6] = np.asarray(inputs["ff_b1"], np.float32)
    bcol[:, 7] = np.asarray(inputs["ff_b2"], np.float32)
    bcol[:, 8] = np.asarray(inputs["bn_g"], np.float32)
    bcol[:, 9] = np.asarray(inputs["bn_b"], np.float32)
    bcol[0:64, 10] = np.asarray(inputs["b3_g"], np.float32)
    bcol[0:64, 11] = np.asarray(inputs["b3_b"], np.float32)
    bcol[0:64, 12] = np.asarray(inputs["b4_g"], np.float32)
    bcol[0:64, 13] = np.asarray(inputs["b4_b"], np.float32)
    im["bcol"] = bcol
    # packed per-(k,l) weight slabs, order: a0 a1 a2 p0 p1 p2 d0 d1 d2
    Wvl = np.concatenate([np.asarray(inputs["Wvla"], np.float32),
                          np.asarray(inputs["Wvlp"], np.float32),
                          np.asarray(inputs["Wvld"], np.float32)], 0)
    im["wv_all"] = np.ascontiguousarray(
        Wvl.transpose(1, 0, 2).reshape(128, 9 * H))
    Wg = np.concatenate([np.asarray(inputs["Wga"], np.float32),
                         np.asarray(inputs["Wgp"], np.float32),
                         np.asarray(inputs["Wgd"], np.float32)], 0)
    im["wi_all"] = np.ascontiguousarray(
        Wg[:, 0:H, :].transpose(1, 0, 2).reshape(128, 9 * H))
    im["wj_all"] = np.ascontiguousarray(
        Wg[:, H:2 * H, :].transpose(1, 0, 2).reshape(128, 9 * H))
    im["we_all"] = np.ascontiguousarray(
        Wg[:, 2 * H:2 * H + HE, :].transpose(1, 0, 2).reshape(64, 9 * H))
    im["ff_w1"] = np.asarray(inputs["ff_w1"], np.float32)
    im["ff_w2"] = np.asarray(inputs["ff_w2"], np.float32)
    return im


def get_in_maps(inputs):
    return [_prep_core(inputs, c) for c in range(NCORE)]


def kernel(**inputs):
    if "nc" not in _CACHE:
        _CACHE["nc"] = build()
    nc = _CACHE["nc"]
    from concourse.bass_utils import run_bass_kernel_spmd
    in_maps = get_in_maps(inputs)
    res = run_bass_kernel_spmd(nc, in_maps, list(range(NCORE))).results
    od = np.concatenate([res[c]["o_d"] for c in range(NCORE)], 0)
    orr = np.concatenate([res[c]["o_r"] for c in range(NCORE)], 0)
    return od, orr


# revision 36
# speedup vs baseline: 1.0050x; 1.0050x over previous
"""Trainium2 Bass kernel for nn_Encoder (GNN message passing, PDP-VRP encoder).

Sharding: 2 graphs per core x 8 cores. Cross-graph row scramble handled with a
ReduceScatter in global-flat row order; BatchNorm stats via moment AllReduce.
Conv in feature-major layout:
  psum[h,(j,i)] = laT.T @ E_sb (+mask fold) + wi.T@x bcast + wj.T@x bcast
  t = prelu(psum + bias) on Act (bias = BN-shift - 200, mask row adds +200)
  P = exp(t) f16; D = sum_i P (DVE fold1+fold2+red); N = sum_i P*x
  (DVE mult, Pool folds, DVE red; N-reductions deferred one group to hide
  the Pool round-trip). d/r chains staggered by half a layer; FF head part-1
  runs inside each chain's stream. E embeddings stay in SBUF (no DRAM
  round-trip); weight loads batched into a few packed DMAs.
"""
import numpy as np

B, D, NN = 16, 2, 100
N2, NA = 50, 102
H, HE, L = 128, 64, 3
SLOPE, EPS = 0.2, 1e-5
NCORE = 8
BL = 2                     # graphs per core
COLS = BL * NA * NA        # 20808 edge cols per chain per core
FLAT = B * (NA + 2 * N2)   # 3232 global flat rows
WIN = FLAT // NCORE        # 404 rows per core window
ECH = 1536                 # embed streaming chunk

_CACHE = {}


def _chunks_full():
    return [(j, 5) for j in range(0, 100, 5)] + [(100, 2)]


def _chunks_sub():
    return [(j, 10) for j in range(0, 50, 10)]


def _groups(chunks, n=3):
    return [chunks[i:i + n] for i in range(0, len(chunks), n)]


def build(emulate_collectives=False):
    import concourse.bass as bass
    import concourse.bacc as bacc
    import concourse.tile as tile
    import concourse.mybir as mybir
    from concourse import masks

    dt = mybir.dt
    F32, F16 = dt.float32, dt.float16
    AF = mybir.ActivationFunctionType
    OP = mybir.AluOpType
    AX = mybir.AxisListType

    nc = bacc.Bacc("TRN2", target_bir_lowering=False, debug=False,
                   num_devices=NCORE)

    def din(name, shape, d=F32):
        return nc.dram_tensor(name, shape, d, kind="ExternalInput").ap()

    dsT = din("dsT", [5, BL * NA])
    pkinT = din("pkinT", [10, BL * N2])
    nat_pack = din("nat_pack", [BL * N2, 23])      # dep|pk|dl natural-layout
    eT = {c: din(f"eT_{c}", [2, COLS], F16) for c in "dr"}
    e_nat = {c: din(f"e_nat_{c}", [128, 163 * 3]) for c in "dr"}
    m_in = {c: din(f"m_{c}", [1, COLS], F16) for c in "dr"}
    W012 = din("W012", [10, 384])                  # W1 | W0 | W2 columns
    W34 = din("W34", [2, 128])                     # W3 | W4
    bcol = din("bcol", [128, 14])                  # packed bias columns
    wv_all = din("wv_all", [128, 9 * H])           # (k,l) major
    wi_all = din("wi_all", [128, 9 * H])
    wj_all = din("wj_all", [128, 9 * H])
    we_all = din("we_all", [64, 9 * H])
    ff_w1 = din("ff_w1", [H, H])
    ff_w2 = din("ff_w2", [H, H])

    o_out = {c: nc.dram_tensor(f"o_{c}", [BL, NA, H], F32,
                               kind="ExternalOutput").ap() for c in "dr"}

    rs_in = {(c, l): nc.dram_tensor(f"rsi_{c}{l}", [FLAT, H], F32).ap()
             for c in "dr" for l in range(L)}
    rs_out = {(c, l): nc.dram_tensor(f"rso_{c}{l}", [WIN, H], F32).ap()
              for c in "dr" for l in range(L)}
    ar1_i = nc.dram_tensor("ar1_i", [128, 16], F32).ap()
    ar1_o = nc.dram_tensor("ar1_o", [128, 16], F32).ap()
    ar2_i = nc.dram_tensor("ar2_i", [128, 8], F32).ap()
    ar2_o = nc.dram_tensor("ar2_o", [128, 8], F32).ap()
    GRP = [list(range(NCORE))]

    import contextlib
    with tile.TileContext(nc) as tc, contextlib.ExitStack() as ctx:
        cpool = ctx.enter_context(tc.tile_pool(name="const", bufs=1))
        wk = ctx.enter_context(tc.tile_pool(name="work", bufs=3))
        xpool = ctx.enter_context(tc.tile_pool(name="xt", bufs=6))
        fpool = ctx.enter_context(tc.tile_pool(name="f16", bufs=3))
        ps_b = ctx.enter_context(tc.tile_pool(name="psb", bufs=2, space="PSUM"))
        ps_s = ctx.enter_context(tc.tile_pool(name="pss", bufs=2, space="PSUM"))

        def ctile(shape, d, tag):
            return cpool.tile(shape, d, tag=tag, name=tag)

        ident = ctile([128, 128], F32, "ident")
        masks.make_identity(nc, ident[:])

        # ---- batched constant loads ----
        W012s = ctile([10, 384], F32, "W012s")
        nc.sync.dma_start(W012s[:], W012[:])
        W1s = W012s[:, 0:128]
        W0s = W012s[0:5, 128:256]
        W2s = W012s[0:5, 256:384]
        W34s = ctile([2, 128], F32, "W34s")
        nc.sync.dma_start(W34s[:], W34[:])
        Wes = {"d": W34s[:, 0:64], "r": W34s[:, 64:128]}
        W34h = ctile([2, 128], F16, "W34h")
        nc.vector.tensor_copy(W34h[:], W34s[:])
        Wesh = {"d": W34h[:, 0:64], "r": W34h[:, 64:128]}
        bcs = ctile([128, 14], F32, "bcs")
        nc.sync.dma_start(bcs[:], bcol[:])
        # col order: b0_g,b0_b,b1_g,b1_b,b2_g,b2_b,ff_b1,ff_b2,bn_g,bn_b,
        #            b3_g,b3_b,b4_g,b4_b (last 4 rows 0:64)
        gcol = {f"b{i}_g": bcs[0:128, 2 * i:2 * i + 1] for i in range(3)}
        gcol.update({f"b{i}_b": bcs[0:128, 2 * i + 1:2 * i + 2]
                     for i in range(3)})
        ffb1c, ffb2c = bcs[:, 6:7], bcs[:, 7:8]
        bngc, bnbc = bcs[:, 8:9], bcs[:, 9:10]
        gcol["b3_g"] = bcs[0:64, 10:11]
        gcol["b3_b"] = bcs[0:64, 11:12]
        gcol["b4_g"] = bcs[0:64, 12:13]
        gcol["b4_b"] = bcs[0:64, 13:14]

        wvs = ctile([128, 9 * H], F32, "wvs")
        nc.sync.dma_start(wvs[:], wv_all[:])
        KIDX = {("a", 0): 0, ("a", 1): 1, ("a", 2): 2,
                ("p", 0): 3, ("p", 1): 4, ("p", 2): 5,
                ("d", 0): 6, ("d", 1): 7, ("d", 2): 8}

        def wsl(t, k, l):
            i = KIDX[(k, l)]
            return t[:, i * H:(i + 1) * H]

        wi_s, wj_s = {}, {}
        for src_dram, dst in ((wi_all, wi_s), (wj_all, wj_s)):
            wtmp = wk.tile([128, 9 * H], F32, tag="wtmp", bufs=1, name="wtmp")
            nc.sync.dma_start(wtmp[:], src_dram[:])
            for k in "apd":
                for l in range(L):
                    nm = "wir" if dst is wi_s else "wjr"
                    tir = ctile([H, H], dt.float32r, f"{nm}{k}{l}")
                    nc.vector.tensor_copy(tir[:], wsl(wtmp, k, l))
                    dst[(k, l)] = tir

        with tc.tile_critical():
            pid = nc.gpsimd.partition_id()

        zero128 = ctile([128, 128], F32, "zero")
        nc.vector.memset(zero128[:], 0.0)
        zrep = zero128[:].unsqueeze(1).broadcast_to([128, 25, 128])
        for key, t in rs_in.items():
            nc.scalar.dma_start(t[0:3200, :].rearrange("(a b) h -> b a h", b=128),
                                zrep)
            nc.scalar.dma_start(t[3200:FLAT, :], zero128[0:32, :])

        # ---------------- embeddings & stats ----------------
        stats = ctile([128, 16], F32, "stats")
        nc.vector.memset(stats[:], 0.0)

        def evac(ps_ap, hh, wid, tag, d=F32):
            t = wk.tile([hh, wid], d, tag=tag, name=tag)
            nc.scalar.copy(t[:], ps_ap)
            return t

        natp = wk.tile([BL * N2, 23], F32, tag="natp", name="natp")
        nc.sync.dma_start(natp[:], nat_pack[:])

        def moments(nat_ap, rows, fdim):
            ps = ps_s.tile([fdim, fdim + 1], F32, tag="pss", name="psm")
            nc.tensor.matmul(ps[:], nat_ap[0:rows, 0:fdim], nat_ap[0:rows, :],
                             start=True, stop=True)
            return evac(ps[:], fdim, fdim + 1, f"mom{fdim}")

        def w_transpose(w, kdim, hh):
            ps = ps_s.tile([hh, kdim], F32, tag="pss", name="pswt")
            nc.tensor.transpose(ps[:], w, ident[:kdim, :kdim])
            return evac(ps[:], hh, kdim, "wT")

        def embed_stats(mom, Ws, kdim, hh, scol):
            ps = ps_s.tile([hh, 1], F32, tag="pss", name="pse1")
            nc.tensor.matmul(ps[:], Ws, mom[:, kdim:kdim + 1],
                             start=True, stop=True)
            nc.vector.tensor_copy(stats[0:hh, scol:scol + 1], ps[:])
            ps2 = ps_s.tile([hh, kdim], F32, tag="pss", name="pse2")
            nc.tensor.matmul(ps2[:], Ws, mom[:, 0:kdim],
                             start=True, stop=True)
            G2 = evac(ps2[:], hh, kdim, "G2")
            WT = w_transpose(Ws, kdim, hh)
            prod = wk.tile([hh, kdim], F32, tag="prod", name="prod")
            nc.vector.tensor_tensor(prod[:], G2[:], WT[:], op=OP.mult)
            nc.vector.tensor_reduce(stats[0:hh, scol + 1:scol + 2], prod[:],
                                    axis=AX.X, op=OP.add)

        # nat_pack: cols 0:6 dep (rows 0:BL*D), 6:17 pk, 17:23 dl
        embed_stats(moments(natp[0:BL * D, 0:6], BL * D, 5), W0s, 5, H, 0)
        embed_stats(moments(natp[:, 6:17], BL * N2, 10), W1s, 10, H, 2)
        embed_stats(moments(natp[:, 17:23], BL * N2, 5), W2s, 5, H, 4)

        for ci, c in enumerate("dr"):
            na = wk.tile([128, 163 * 3], F32, tag="enat", bufs=1, name="enat")
            nc.sync.dma_start(na[:], e_nat[c][:])
            nav = na[:].rearrange("p (n c) -> p n c", n=163)
            ps = ps_s.tile([2, 3], F32, tag="pss", name="psen")
            for n in range(163):
                nc.tensor.matmul(ps[:], nav[:, n, 0:2], nav[:, n, :],
                                 start=(n == 0), stop=(n == 162))
            mom = evac(ps[:], 2, 3, "mome")
            embed_stats(mom, Wes[c], 2, HE, 6 + 2 * ci)

        # ---- edge embedding into SBUF: E_sb rows 0..63 raw z; row 64 mask ----
        E_sb = {}
        for ei, c in enumerate("dr"):
            Et = cpool.tile([65, COLS], F16, tag=f"Esb{c}", name=f"Esb{c}")
            E_sb[c] = Et
            EQ = 2 * ECH  # 3072: eT staging block, multiple of ECH
            ets = None
            for ci2, c0 in enumerate(range(0, COLS, ECH)):
                CH = min(ECH, COLS - c0)
                if c0 % EQ == 0:
                    ets = wk.tile([2, EQ], F16, tag="ets", bufs=2, name="ets")
                    nc.sync.dma_start(ets[:, 0:min(EQ, COLS - c0)],
                                      eT[c][:, c0:c0 + min(EQ, COLS - c0)])
                eo = c0 % EQ
                psg = ps_b.tile([128, 1536], F32, tag="psg", name="psge")
                for k in range((CH + 511) // 512):
                    w = min(512, CH - k * 512)
                    nc.tensor.matmul(psg[0:64, k * 512:k * 512 + w],
                                     Wesh[c],
                                     ets[:, eo + k * 512:eo + k * 512 + w],
                                     start=True, stop=True)
                if ci2 % 2 == 0:
                    nc.scalar.copy(Et[0:64, c0:c0 + CH], psg[0:64, 0:CH])
                else:
                    nc.vector.tensor_copy(Et[0:64, c0:c0 + CH],
                                          psg[0:64, 0:CH])
            nc.sync.dma_start(Et[64:65, :], m_in[c][:])

        nc.sync.dma_start(ar1_i[:], stats[:])
        if emulate_collectives:
            nc.sync.dma_start(ar1_o[:], ar1_i[:])
        else:
            nc.gpsimd.collective_compute("AllReduce", OP.add, replica_groups=GRP,
                                         ins=[ar1_i], outs=[ar1_o])
        sts = ctile([128, 16], F32, "sts")
        nc.sync.dma_start(sts[:], ar1_o[:])

        def bn_vecs(src, scol, n, gc, bc, hh, tag):
            inv = 1.0 / n
            m = wk.tile([hh, 1], F32, tag=f"m{tag}", name=f"m{tag}")
            nc.vector.tensor_scalar_mul(m[:], src[0:hh, scol:scol + 1], inv)
            v = wk.tile([hh, 1], F32, tag=f"v{tag}", name=f"v{tag}")
            nc.vector.tensor_scalar_mul(v[:], src[0:hh, scol + 1:scol + 2], inv)
            msq = wk.tile([hh, 1], F32, tag=f"q{tag}", name=f"q{tag}")
            nc.vector.tensor_tensor(msq[:], m[:], m[:], op=OP.mult)
            nc.vector.tensor_tensor(v[:], v[:], msq[:], op=OP.subtract)
            nc.vector.tensor_scalar_add(v[:], v[:], EPS)
            sd = wk.tile([hh, 1], F32, tag=f"s{tag}", name=f"s{tag}")
            nc.scalar.activation(sd[:], v[:], AF.Sqrt)
            rsd = wk.tile([hh, 1], F32, tag=f"r{tag}", name=f"r{tag}")
            nc.vector.reciprocal(rsd[:], sd[:])
            sc = ctile([hh, 1], F32, f"sc{tag}")
            nc.vector.tensor_tensor(sc[:], rsd[:], gc, op=OP.mult)
            sh = ctile([hh, 1], F32, f"sh{tag}")
            nc.vector.tensor_tensor(sh[:], m[:], sc[:], op=OP.mult)
            nc.vector.tensor_tensor(sh[:], bc, sh[:], op=OP.subtract)
            return sc, sh

        sc0, sh0 = bn_vecs(sts, 0, B * D, gcol["b0_g"], gcol["b0_b"], H, "b0")
        sc1, sh1 = bn_vecs(sts, 2, B * N2, gcol["b1_g"], gcol["b1_b"], H, "b1")
        sc2, sh2 = bn_vecs(sts, 4, B * N2, gcol["b2_g"], gcol["b2_b"], H, "b2")
        sce, she = {}, {}
        sce["d"], she["d"] = bn_vecs(sts, 6, B * NA * NA, gcol["b3_g"],
                                     gcol["b3_b"], HE, "b3")
        sce["r"], she["r"] = bn_vecs(sts, 8, B * NA * NA, gcol["b4_g"],
                                     gcol["b4_b"], HE, "b4")

        # lhsT (65 rows: 64 e-rows scaled by BN, row 64 = +200 mask weight),
        # bias column = we.T @ she - 200, and its f16 row for the K=1 matmul.
        wes_f = wk.tile([64, 9 * H], F32, tag="wtmp", bufs=1, name="wes_f")
        nc.sync.dma_start(wes_f[:], we_all[:])
        lhsT_aug, bias_col = {}, {}
        for c in "dr":
            for k in "apd":
                for l in range(L):
                    t = ctile([65, H], F16, f"la{c}{k}{l}")
                    nc.vector.tensor_scalar(t[0:64, :], wsl(wes_f, k, l),
                                            sce[c][:], None, op0=OP.mult)
                    nc.vector.memset(t[64:65, :], 200.0)
                    lhsT_aug[(c, k, l)] = t
                    ps = ps_s.tile([H, 1], F32, tag="pss", name="pscc")
                    nc.tensor.matmul(ps[:], wsl(wes_f, k, l), she[c][:],
                                     start=True, stop=True)
                    bc = ctile([H, 1], F32, f"bc{c}{k}{l}")
                    nc.vector.tensor_scalar_add(bc[:], ps[:], -200.0)
                    bias_col[(c, k, l)] = bc

        # node embeddings -> xT0
        xT0 = xpool.tile([H, BL * NA], F32, tag="xT", name="xT0")
        dsTs = wk.tile([5, BL * NA], F32, tag="dsTs", name="dsTs")
        nc.sync.dma_start(dsTs[:], dsT[:])
        dsv = dsTs[:].rearrange("p (g n) -> p g n", g=BL)
        x0v = xT0[:].rearrange("p (g n) -> p g n", g=BL)
        ps = ps_s.tile([H, BL * D], F32, tag="pss", name="psx0")
        nc.tensor.matmul(ps[:], W0s, dsv[:, :, 0:D], start=True, stop=True)
        nc.vector.tensor_scalar(
            x0v[:, :, 0:D], ps[:].rearrange("p (g n) -> p g n", g=BL),
            sc0[:], sh0[:], op0=OP.mult, op1=OP.add)
        pkt = wk.tile([10, BL * N2], F32, tag="pkt", name="pkt")
        nc.sync.dma_start(pkt[:], pkinT[:])
        ps = ps_s.tile([H, BL * N2], F32, tag="pss", name="psx1")
        nc.tensor.matmul(ps[:], W1s, pkt[:], start=True, stop=True)
        nc.vector.tensor_scalar(
            x0v[:, :, D:D + N2], ps[:].rearrange("p (g n) -> p g n", g=BL),
            sc1[:], sh1[:], op0=OP.mult, op1=OP.add)
        ps = ps_s.tile([H, BL * N2], F32, tag="pss", name="psx2")
        nc.tensor.matmul(ps[:], W2s, dsv[:, :, D + N2:NA],
                         start=True, stop=True)
        nc.vector.tensor_scalar(
            x0v[:, :, D + N2:NA], ps[:].rearrange("p (g n) -> p g n", g=BL),
            sc2[:], sh2[:], op0=OP.mult, op1=OP.add)

        # ---------------- conv layers ----------------
        grp_ctr = [0]
        unit_ctr = [0]
        defq = []   # deferred-by-one-group ops (cross-engine latency hiding)

        def flush_defq():
            for fn in defq:
                fn()
            defq.clear()

        def mm_evac(w, rhs_ap, wid, tag):
            ps = ps_s.tile([H, wid], F32, tag="pss", name="psmm")
            nc.tensor.matmul(ps[:], w, rhs_ap, start=True, stop=True)
            t = xpool.tile([H, wid], dt.float32r, tag=tag, bufs=2, name=tag)
            nc.scalar.copy(t[:], ps[:])
            th = xpool.tile([H, wid], F16, tag=tag + "h", bufs=2,
                            name=tag + "h")
            nc.vector.tensor_copy(th[:], ps[:])
            return t, th

        def conv_units(c, l, xTin):
            xv = xTin[:].rearrange("p (g n) -> p g n", g=BL)
            xall, xallh = mm_evac(wsl(wvs, "a", l), xTin[:], BL * NA, "xa")
            pick, pickh = mm_evac(wsl(wvs, "p", l), xv[:, :, D:D + N2],
                                  BL * N2, "xp")
            deli, delih = mm_evac(wsl(wvs, "d", l), xv[:, :, D + N2:NA],
                                  BL * N2, "xd")
            Ev = E_sb[c][:].rearrange("p (g j i) -> p g j i", g=BL, j=NA)
            cfg = [("a", xall, xallh, NA, _chunks_full()),
                   ("p", pick, pickh, N2, _chunks_sub()),
                   ("d", deli, delih, N2, _chunks_sub())]
            units = []
            for k_, xk_, xkh_, S_, chunks_ in cfg:
                for g_ in range(BL):
                    units.append((k_, xk_, xkh_, S_, chunks_, g_))

            def emit_unit(u):
                k, xk, xkh, S, chunks, g = u
                ilen = S
                la = lhsT_aug[(c, k, l)]
                bc = bias_col[(c, k, l)]
                wi, wjt = wi_s[(k, l)], wj_s[(k, l)]
                if k == "a":
                    esrc = Ev[0:65, g]            # [65, NA, NA]
                elif k == "p":
                    esrc = Ev[0:65, g, D:D + N2, D:D + N2]
                else:
                    esrc = Ev[0:65, g, D + N2:NA, D + N2:NA]
                Dt = wk.tile([H, S], F16, tag="Dt", name="Dt")
                Nt = wk.tile([H, S], F16, tag="Nt", name="Nt")
                for grp in _groups(chunks):
                    gi = grp_ctr[0]
                    grp_ctr[0] += 1
                    flush_defq()   # prior group's deferred reds first in queue
                    psg = ps_b.tile([128, 1536], F32, tag="psg", name="psg")
                    for ki, (j0, nj) in enumerate(grp):
                        nc.tensor.matmul(
                            psg[:, ki * 512:ki * 512 + nj * ilen],
                            la[:], esrc[:, j0:j0 + nj, :],
                            start=True, stop=False)
                    for ki, (j0, nj) in enumerate(grp):
                        a_rhs = xk[:, g * S + j0:g * S + j0 + nj]\
                            .unsqueeze(2).broadcast_to([H, nj, ilen])
                        nc.tensor.matmul(
                            psg[:, ki * 512:ki * 512 + nj * ilen],
                            wi[:], a_rhs, start=False, stop=False)
                    b_base = xk[:, g * S:g * S + ilen]
                    for ki, (j0, nj) in enumerate(grp):
                        b_rhs = b_base.unsqueeze(1)\
                            .broadcast_to([H, nj, ilen])
                        nc.tensor.matmul(
                            psg[:, ki * 512:ki * 512 + nj * ilen],
                            wjt[:], b_rhs, start=False, stop=True)
                    fd = (len(grp) - 1) * 512 + grp[-1][1] * ilen
                    tg = fpool.tile([128, 1536], F16, tag="tg", bufs=3,
                                    name="tg")
                    nc.scalar.activation(tg[:, 0:fd], psg[:, 0:fd],
                                         AF.Prelu, bias=bc[:], alpha=SLOPE)
                    Pg = fpool.tile([128, 1536], F16, tag="Pg", bufs=3, name="Pg")
                    nc.scalar.activation(Pg[:, 0:fd], tg[:, 0:fd], AF.Exp)
                    wg = fpool.tile([128, 1536], F16, tag="wg", bufs=3,
                                    name="wg")
                    xkb = xkh[:, g * S:g * S + ilen]
                    runs = []
                    for ki, (j0, nj) in enumerate(grp):
                        if runs and runs[-1][2] == nj:
                            runs[-1][1] += 1
                        else:
                            runs.append([ki, 1, nj, j0])
                    hl = ilen // 2          # 51 or 25
                    ql = hl // 2            # 25 or 12
                    pf = fpool.tile([128, 768], F16, tag="pf", bufs=3,
                                    name="pf")
                    wf = fpool.tile([128, 768], F16, tag="wf", bufs=3,
                                    name="wf")
                    pq = fpool.tile([128, 384], F16, tag="pq", bufs=3,
                                    name="pq")
                    wq = fpool.tile([128, 384], F16, tag="wq", bufs=3,
                                    name="wq")
                    reds = []
                    for k0, nk, nj, j0r in runs:
                        base = Pg[:, k0 * 512:(k0 + nk) * 512]
                        wbase = wg[:, k0 * 512:(k0 + nk) * 512]
                        pv4 = base.rearrange("p (k r) -> p k r", k=nk)\
                            [:, :, 0:nj * ilen].rearrange(
                            "p k (j i) -> p k j i", i=ilen)
                        wv4 = wbase.rearrange("p (k r) -> p k r", k=nk)\
                            [:, :, 0:nj * ilen].rearrange(
                            "p k (j i) -> p k j i", i=ilen)
                        xb4 = xkb.unsqueeze(1).unsqueeze(1)\
                            .broadcast_to([H, nk, nj, ilen])
                        nc.vector.tensor_tensor(wv4, pv4, xb4, op=OP.mult)
                        pf4 = pf[:, k0 * 256:k0 * 256 + nk * nj * hl]\
                            .rearrange("p (k j i) -> p k j i", k=nk, j=nj)
                        wf4 = wf[:, k0 * 256:k0 * 256 + nk * nj * hl]\
                            .rearrange("p (k j i) -> p k j i", k=nk, j=nj)
                        # fold1: i halves
                        nc.vector.tensor_tensor(
                            pf4, pv4[:, :, :, 0:hl],
                            pv4[:, :, :, hl:2 * hl], op=OP.add)
                        nc.gpsimd.tensor_tensor(
                            wf4, wv4[:, :, :, 0:hl],
                            wv4[:, :, :, hl:2 * hl], op=OP.add)
                        # fold2: halves of hl (ql cols), leftover col if odd
                        pq4 = pq[:, k0 * 128:k0 * 128 + nk * nj * ql]\
                            .rearrange("p (k j i) -> p k j i", k=nk, j=nj)
                        wq4 = wq[:, k0 * 128:k0 * 128 + nk * nj * ql]\
                            .rearrange("p (k j i) -> p k j i", k=nk, j=nj)
                        nc.vector.tensor_tensor(
                            pq4, pf4[:, :, :, 0:ql],
                            pf4[:, :, :, ql:2 * ql], op=OP.add)
                        nc.gpsimd.tensor_tensor(
                            wq4, wf4[:, :, :, 0:ql],
                            wf4[:, :, :, ql:2 * ql], op=OP.add)
                        dv = Dt[:, j0r:j0r + nk * nj].rearrange(
                            "p (k j) -> p k j", k=nk)
                        nv = Nt[:, j0r:j0r + nk * nj].rearrange(
                            "p (k j) -> p k j", k=nk)
                        with nc.allow_low_precision("fp16 softmax sums"):
                            nc.vector.tensor_reduce(dv, pq4, axis=AX.X,
                                                    op=OP.add)
                            if 2 * ql != hl:
                                nc.vector.tensor_tensor(
                                    dv, dv, pf4[:, :, :, 2 * ql], op=OP.add)

                        def red_n(nv=nv, wq4=wq4, wf4=wf4, ql=ql, hl=hl):
                            with nc.allow_low_precision("fp16 softmax sums"):
                                nc.vector.tensor_reduce(nv, wq4, axis=AX.X,
                                                        op=OP.add)
                                if 2 * ql != hl:
                                    nc.vector.tensor_tensor(
                                        nv, nv, wf4[:, :, :, 2 * ql],
                                        op=OP.add)
                        reds.append(red_n)
                    defq.extend(reds)

                def epilogue(k=k, g=g, S=S, Dt=Dt, Nt=Nt):
                    Di = wk.tile([H, S], F32, tag="Di", name="Di")
                    nc.vector.reciprocal(Di[:], Dt[:])
                    og = wk.tile([H, S], F32, tag="og", name="og")
                    nc.vector.tensor_tensor(og[:], Nt[:], Di[:], op=OP.mult)
                    pst = ps_s.tile([S, H], F32, tag="pss", name="psout")
                    nc.tensor.transpose(pst[:], og[:], ident[:])
                    nat2 = wk.tile([S, H], F32, tag="nat2", name="nat2")
                    nc.vector.tensor_copy(nat2[:], pst[:])
                    unit_ctr[0] += 1
                    if k == "a":
                        off = pid * (BL * NA) + g * NA
                    elif k == "p":
                        off = pid * (BL * N2) + g * N2 + B * NA
                    else:
                        off = pid * (BL * N2) + g * N2 + B * (NA + N2)
                    nc.gpsimd.dma_start(rs_in[(c, l)][bass.ds(off, S), :],
                                        nat2[:])
                defq.append(epilogue)

            def emit_rs():
                flush_defq()
                if emulate_collectives:
                    nc.sync.dma_start(rs_out[(c, l)][:], rs_in[(c, l)][0:WIN, :])
                else:
                    nc.gpsimd.collective_compute(
                        "ReduceScatter", OP.add, replica_groups=GRP,
                        ins=[rs_in[(c, l)]], outs=[rs_out[(c, l)]])
            return [lambda u=u: emit_unit(u) for u in units] + [emit_rs]

        def assemble(c, l):
            xn = xpool.tile([H, BL * NA], F32, tag="xT", name="xn")
            for g in range(BL):
                n1 = wk.tile([128, H], F32, tag="asm", name="asm1")
                nc.sync.dma_start(n1[:],
                                  rs_out[(c, l)][g * 202:g * 202 + 128, :])
                n2 = wk.tile([128, H], F32, tag="asm", name="asm2")
                nc.sync.dma_start(
                    n2[0:74, :], rs_out[(c, l)][g * 202 + 128:g * 202 + 202, :])
                p1 = ps_s.tile([H, 128], F32, tag="pss", name="psa1")
                nc.tensor.transpose(p1[:], n1[:], ident[:])
                p2 = ps_s.tile([H, 128], F32, tag="pss", name="psa2")
                nc.tensor.transpose(p2[:, 0:74], n2[0:74, :],
                                    ident[0:74, 0:74])
                # cols: 0:102 xa | 102:202 xp,xd -> add onto nodes 2..102
                xb1 = wk.tile([H, 128], F32, tag="xb1", name="xb1")
                nc.vector.tensor_copy(xb1[:], p1[:])
                nc.vector.tensor_copy(xn[:, g * NA:g * NA + D], xb1[:, 0:D])
                nc.vector.tensor_tensor(xn[:, g * NA + D:g * NA + D + 26],
                                        xb1[:, D:D + 26], xb1[:, 102:128],
                                        op=OP.add)
                nc.vector.tensor_tensor(xn[:, g * NA + D + 26:g * NA + NA],
                                        xb1[:, D + 26:NA], p2[:, 0:74],
                                        op=OP.add)
            return xn

        # FF part-1 consts (loaded early so stream FF thunks can run)
        ffw1 = ctile([H, H], F32, "ffw1")
        nc.sync.dma_start(ffw1[:], ff_w1[:])
        ffw2 = ctile([H, H], F32, "ffw2")
        nc.sync.dma_start(ffw2[:], ff_w2[:])
        zt = {}
        st2 = ctile([128, 8], F32, "st2")
        nc.vector.memset(st2[:], 0.0)

        def ff_part1(c):
            ci = 0 if c == "d" else 1
            ps1 = ps_s.tile([H, BL * NA], F32, tag="pss", name="psf1")
            nc.tensor.matmul(ps1[:], ffw1[:], xT[c][:], start=True, stop=True)
            r = wk.tile([H, BL * NA], F32, tag="ffr", name="ffr")
            nc.scalar.activation(r[:], ps1[:], AF.Relu, bias=ffb1c)
            ps2 = ps_s.tile([H, BL * NA], F32, tag="pss", name="psf2")
            nc.tensor.matmul(ps2[:], ffw2[:], r[:], start=True, stop=True)
            z = xpool.tile([H, BL * NA], F32, tag="zt", bufs=2, name="zt")
            nc.vector.scalar_tensor_tensor(z[:], ps2[:], ffb2c, xT[c][:],
                                           op0=OP.add, op1=OP.add)
            zt[c] = z
            nc.vector.tensor_reduce(st2[:, 2 * ci:2 * ci + 1], z[:],
                                    axis=AX.X, op=OP.add)
            sq = fpool.tile([H, BL * NA], F16, tag="sq", name="sq")
            nc.scalar.activation(sq[:], z[:], AF.Square,
                                 accum_out=st2[:, 2 * ci + 1:2 * ci + 2])

        xT = {"d": xT0, "r": xT0}

        def stream(c):
            # per layer: prep(mm_evac) + 6 units + rs + assemble = 9 thunks
            for l in range(L):
                holder = {}

                def prep(l=l, c=c, holder=holder):
                    holder["units"] = conv_units(c, l, xT[c])
                yield prep
                for i in range(7):
                    yield (lambda i=i, holder=holder: holder["units"][i]())
                yield (lambda c=c, l=l: xT.__setitem__(c, assemble(c, l)))
            yield (lambda c=c: ff_part1(c))

        sd, sr = stream("d"), stream("r")
        for _ in range(5):          # stagger chains by ~half a layer
            next(sd)()
        nd = nr = False
        while not (nd and nr):
            if not nd:
                try:
                    next(sd)()
                except StopIteration:
                    nd = True
            if not nr:
                try:
                    next(sr)()
                except StopIteration:
                    nr = True

        # ---------------- FF head (part 2: stats exchange + BN) ---------
        nc.sync.dma_start(ar2_i[:], st2[:])
        if emulate_collectives:
            nc.sync.dma_start(ar2_o[:], ar2_i[:])
        else:
            nc.gpsimd.collective_compute("AllReduce", OP.add, replica_groups=GRP,
                                         ins=[ar2_i], outs=[ar2_o])
        st2o = ctile([128, 8], F32, "st2o")
        nc.sync.dma_start(st2o[:], ar2_o[:])
        for ci, c in enumerate("dr"):
            sc, sh = bn_vecs(st2o, 2 * ci, B * NA, bngc, bnbc, H, f"ff{ci}")
            oT = wk.tile([H, BL * NA], F32, tag="oT", name="oT")
            nc.vector.tensor_scalar(oT[:], zt[c][:], sc[:], sh[:],
                                    op0=OP.mult, op1=OP.add)
            for g in range(BL):
                pso = ps_s.tile([NA, H], F32, tag="pss", name="pso")
                nc.tensor.transpose(pso[:], oT[:, g * NA:(g + 1) * NA],
                                    ident[:])
                on = wk.tile([NA, H], F32, tag="on", name="on")
                nc.scalar.copy(on[:], pso[:])
                nc.sync.dma_start(o_out[c][g], on[:])

    nc.compile()
    return nc


def _prep_core(inputs, c):
    sl = slice(2 * c, 2 * c + 2)
    x = np.asarray(inputs["x"])[sl]
    dem = np.asarray(inputs["demand"])[sl]
    tw = np.asarray(inputs["time_window"])[sl]
    ds = np.concatenate([x, dem, tw], -1).astype(np.float32)
    dsT = np.ascontiguousarray(ds.transpose(2, 0, 1).reshape(5, BL * NA))
    pkin = np.concatenate([ds[:, D:D + N2], ds[:, D + N2:NA]], -1)
    pkinT = np.ascontiguousarray(pkin.transpose(2, 0, 1).reshape(10, BL * N2))
    nat_pack = np.ones((BL * N2, 23), np.float32)
    nat_pack[:, 0:6] = 1.0
    nat_pack[:BL * D, 0:5] = ds[:, :D].reshape(BL * D, 5)
    nat_pack[BL * D:, 0:6] = 0.0
    nat_pack[:, 6:16] = pkin.reshape(BL * N2, 10)
    nat_pack[:, 16] = 1.0
    nat_pack[:, 17:22] = ds[:, D + N2:NA].reshape(BL * N2, 5)
    nat_pack[:, 22] = 1.0
    im = dict(dsT=dsT, pkinT=pkinT, nat_pack=nat_pack)
    for c2, key_e, key_m in (("d", "edge_attr_d", "mask_adjacency_d"),
                             ("r", "edge_attr_r", "mask_adjacency_r")):
        ea = np.asarray(inputs[key_e])[sl].reshape(BL, NA, NA, 2)
        im[f"eT_{c2}"] = np.ascontiguousarray(
            ea.transpose(3, 0, 2, 1).reshape(2, COLS)).astype(np.float16)
        tmp = np.zeros((163 * 128, 3), np.float32)
        tmp[:BL * NA * NA, :2] = ea.reshape(BL * NA * NA, 2)
        tmp[:BL * NA * NA, 2] = 1.0
        im[f"e_nat_{c2}"] = np.ascontiguousarray(
            tmp.reshape(163, 128, 3).transpose(1, 0, 2).reshape(128, 489))
        mm = np.asarray(inputs[key_m])[sl].reshape(BL, NA, NA)
        im[f"m_{c2}"] = np.ascontiguousarray(
            mm.transpose(0, 2, 1).reshape(1, COLS)).astype(np.float16)
    W0 = np.asarray(inputs["W0"], np.float32)
    W1 = np.asarray(inputs["W1"], np.float32)
    W2 = np.asarray(inputs["W2"], np.float32)
    W012 = np.zeros((10, 384), np.float32)
    W012[0:10, 0:128] = W1
    W012[0:5, 128:256] = W0
    W012[0:5, 256:384] = W2
    im["W012"] = W012
    W34 = np.zeros((2, 128), np.float32)
    W34[:, 0:64] = np.asarray(inputs["W3"], np.float32)
    W34[:, 64:128] = np.asarray(inputs["W4"], np.float32)
    im["W34"] = W34
    bcol = np.zeros((128, 14), np.float32)
    for i in range(3):
        bcol[:, 2 * i] = np.asarray(inputs[f"b{i}_g"], np.float32)
        bcol[:, 2 * i + 1] = np.asarray(inputs[f"b{i}_b"], np.float32)
    bcol[:, 6] = np.asarray(inputs["ff_b1"], np.float32)
    bcol[:, 7] = np.asarray(inputs["ff_b2"], np.float32)
    bcol[:, 8] = np.asarray(inputs["bn_g"], np.float32)
    bcol[:, 9] = np.asarray(inputs["bn_b"], np.float32)
    bcol[0:64, 10] = np.asarray(inputs["b3_g"], np.float32)
    bcol[0:64, 11] = np.asarray(inputs["b3_b"], np.float32)
    bcol[0:64, 12] = np.asarray(inputs["b4_g"], np.float32)
    bcol[0:64, 13] = np.asarray(inputs["b4_b"], np.float32)
    im["bcol"] = bcol
    # packed per-(k,l) weight slabs, order: a0 a1 a2 p0 p1 p2 d0 d1 d2
    Wvl = np.concatenate([np.asarray(inputs["Wvla"], np.float32),
                          np.asarray(inputs["Wvlp"], np.float32),
                          np.asarray(inputs["Wvld"], np.float32)], 0)
    im["wv_all"] = np.ascontiguousarray(
        Wvl.transpose(1, 0, 2).reshape(128, 9 * H))
    Wg = np.concatenate([np.asarray(inputs["Wga"], np.float32),
                         np.asarray(inputs["Wgp"], np.float32),
                         np.asarray(inputs["Wgd"], np.float32)], 0)
    im["wi_all"] = np.ascontiguousarray(
        Wg[:, 0:H, :].transpose(1, 0, 2).reshape(128, 9 * H))
    im["wj_all"] = np.ascontiguousarray(
        Wg[:, H:2 * H, :].transpose(1, 0, 2).reshape(128, 9 * H))
    im["we_all"] = np.ascontiguousarray(
        Wg[:, 2 * H:2 * H + HE, :].transpose(1, 0, 2).reshape(64, 9 * H))
    im["ff_w1"] = np.asarray(inputs["ff_w1"], np.float32)
    im["ff_w2"] = np.asarray(inputs["ff_w2"], np.float32)
    return im


def get_in_maps(inputs):
    return [_prep_core(inputs, c) for c in range(NCORE)]


def kernel(**inputs):
    if "nc" not in _CACHE:
        _CACHE["nc"] = build()
    nc = _CACHE["nc"]
    from concourse.bass_utils import run_bass_kernel_spmd
    in_maps = get_in_maps(inputs)
    res = run_bass_kernel_spmd(nc, in_maps, list(range(NCORE))).results
    od = np.concatenate([res[c]["o_d"] for c in range(NCORE)], 0)
    orr = np.concatenate([res[c]["o_r"] for c in range(NCORE)], 0)
    return od, orr
